# revision 3
# baseline (speedup 1.0000x reference)
"""Trainium2 Bass kernel for AttentionalPlanarRemapping.

out[n,c,h,w] = sum_d softmax(atts[n,c,:])[d] * images[n,d,h,w]

Per-sample: W = softmax(atts[n]) [C,C]; out[n] = W @ images[n].reshape(C, H*W).

Sharding: data-parallel over N across 8 cores (4 samples per core).

Host preprocessing inside kernel(): atts is uploaded fp16 in BOTH layouts,
packed so every DRAM row holds all 4 of the core's samples (4KB rows ->
full-size DMA packets):
  attsT[d, n, c] = atts[n, c, d]   (contraction d on partitions: matmul lhsT)
  attsC[c, n, d] = atts[n, c, d]   (channel c on partitions: denominator)
Both land once in resident SBUF tiles. Per sample, E_dc = exp(attsT slice)
feeds the matmuls; E_cd = exp(attsC slice) is reduced over its free axis
(DVE) giving the softmax denominators s[c] directly in per-partition
layout -- no transposition, no PE work for the denominator at all. The
fp16 inputs of both exps are bitwise identical, so normalization is
exactly consistent with the matmul weights.

Per-core pipeline (prep(n+1) emitted before compute(n)):
  prep(n):   DMA images[n] -> X [128, KD, 1024] fp16 (atts already resident)
             E_dc = exp(attsT[:,:,n,:]); E_cd = exp(attsC[:,:,n,:]) (ACT)
             s = reduce_add(E_cd, axis=free) [128, KC]; r = 1/s  (DVE)
  compute(n): for kc: ps[128,1024] (PSUM, 2 banks, pool depth 4):
             8 matmuls (4 kd x 2 halves of HW); evict o = ps * r[:,kc]
             -> fp16 (DVE for kc<3, ACT for kc==3); store per kc on
             alternating SWDGE (gpsimd) / HWDGE (scalar) queues.
Sample 0 uses per-kd chunked DMAs + exps so the first matmul starts as
soon as the first 128 attsT rows land; the last sample's final band is
evicted and stored in halves on both engines/queues to shrink the tail.
"""

import numpy as np
from contextlib import ExitStack

import concourse.bass as bass
import concourse.mybir as mybir
import concourse.tile as tile
from concourse import bacc
from concourse.bass_utils import run_bass_kernel_spmd

N, C, H, W = 32, 512, 32, 32
HW = H * W                      # 1024
NCORES = 8
NPC = N // NCORES               # 4 samples per core
P = 128
KC = C // P                     # 4 chunks over output channel c
KD = C // P                     # 4 chunks over contraction d
NT = 512                        # matmul moving free dim (one PSUM bank of f32)
NHT = HW // NT                  # 2

F32 = mybir.dt.float32
F16 = mybir.dt.float16
AF = mybir.ActivationFunctionType
AX = mybir.AxisListType
ALU = mybir.AluOpType


def build_nc():
    nc = bacc.Bacc("TRN2", target_bir_lowering=False, debug=False)

    images = nc.dram_tensor("images", [NPC, C, HW], F16, kind="ExternalInput").ap()
    attsT = nc.dram_tensor("attsT", [C, NPC, C], F16, kind="ExternalInput").ap()
    attsC = nc.dram_tensor("attsC", [C, NPC, C], F16, kind="ExternalInput").ap()
    out = nc.dram_tensor("out", [NPC, C, HW], F16, kind="ExternalOutput").ap()

    with ExitStack() as ctx:
        tc = ctx.enter_context(tile.TileContext(nc))

        aT_pool = ctx.enter_context(tc.tile_pool(name="aT", bufs=1))
        aC_pool = ctx.enter_context(tc.tile_pool(name="aC", bufs=1))
        e_pool = ctx.enter_context(tc.tile_pool(name="e", bufs=2))
        ec_pool = ctx.enter_context(tc.tile_pool(name="ec", bufs=2))
        x_pool = ctx.enter_context(tc.tile_pool(name="x", bufs=2))
        o_pool = ctx.enter_context(tc.tile_pool(name="o", bufs=8))
        s_pool = ctx.enter_context(tc.tile_pool(name="s", bufs=2))
        r_pool = ctx.enter_context(tc.tile_pool(name="r", bufs=2))
        mm_psum = ctx.enter_context(tc.tile_pool(name="mmp", bufs=4, space="PSUM"))

        # resident atts tiles, loaded once; 4KB DRAM rows (all samples packed)
        aT_t = aT_pool.tile([P, KD, NPC, C], F16)
        aC_t = aC_pool.tile([P, KC, NPC, C], F16)

        def load_atts_and_x0(x_t):
            # interleave attsT / images[0] chunks so sample 0's first
            # matmul starts as soon as the first 128 rows of each land
            for kd in range(KD):
                nc.sync.dma_start(aT_t[:, kd], attsT[kd * P : (kd + 1) * P])
                nc.sync.dma_start(
                    x_t[:, kd], images[0][kd * P : (kd + 1) * P]
                )
            nc.sync.dma_start(aC_t[:], attsC.rearrange("(kc p) n d -> p kc n d", p=P))

        def prep(n, x_t=None, fine=False):
            """exp + denominator for sample n (one sample ahead of compute)."""
            if x_t is None:
                x_t = x_pool.tile([P, KD, HW], F16, name=f"x{n}", tag="x")
                nc.sync.dma_start(
                    x_t[:], images[n].rearrange("(kd p) f -> p kd f", p=P)
                )
            e_t = e_pool.tile([P, KD, C], F16, name=f"e{n}", tag="e")
            if fine:
                for kd in range(KD):
                    nc.scalar.activation(
                        e_t[:, kd], aT_t[:, kd, n], AF.Exp, bias=0.0, scale=1.0
                    )
            else:
                nc.scalar.activation(
                    e_t[:], aT_t[:, :, n], AF.Exp, bias=0.0, scale=1.0
                )
            ec_t = ec_pool.tile([P, KC, C], F16, name=f"ec{n}", tag="ec")
            nc.scalar.activation(
                ec_t[:], aC_t[:, :, n], AF.Exp, bias=0.0, scale=1.0
            )
            s_t = s_pool.tile([P, KC], F32, name=f"s{n}", tag="s")
            nc.vector.tensor_reduce(s_t[:], ec_t[:], axis=AX.X, op=ALU.add)
            r_t = r_pool.tile([P, KC], F32, name=f"r{n}", tag="r")
            nc.vector.reciprocal(r_t[:], s_t[:])
            return e_t, x_t, r_t

        def compute(n, e_t, x_t, r_t, last=False):
            for kc in range(KC):
                ps = mm_psum.tile(
                    [P, HW], F32, name=f"ps{n}_{kc}", tag="ps", space="PSUM"
                )
                for kd in range(KD):
                    lhs = e_t[:, kd, kc * P : (kc + 1) * P]
                    for ht in range(NHT):
                        nc.tensor.matmul(
                            ps[:, ht * NT : (ht + 1) * NT],
                            lhsT=lhs,
                            rhs=x_t[:, kd, ht * NT : (ht + 1) * NT],
                            start=(kd == 0),
                            stop=(kd == KD - 1),
                        )
                r_ap = r_t[:, kc : kc + 1]
                o_t = o_pool.tile([P, HW], F16, name=f"o{n}_{kc}", tag="o")
                dst = out[n][kc * P : (kc + 1) * P]
                if last and kc == KC - 1:
                    # tail: evict + store the final band in halves on both
                    # engines/queues so the kernel end isn't serialized
                    # behind one full-width eviction
                    nc.scalar.mul(o_t[:, 0:NT], ps[:, 0:NT], r_ap)
                    nc.vector.tensor_scalar_mul(o_t[:, NT:HW], ps[:, NT:HW], r_ap)
                    nc.gpsimd.dma_start(dst[:, 0:NT], o_t[:, 0:NT])
                    nc.scalar.dma_start(dst[:, NT:HW], o_t[:, NT:HW])
                else:
                    if kc == KC - 1:
                        nc.scalar.mul(o_t[:], ps[:], r_ap)
                    else:
                        nc.vector.tensor_scalar_mul(o_t[:], ps[:], r_ap)
                    if kc % 2 == 0:
                        nc.gpsimd.dma_start(dst, o_t[:])
                    else:
                        nc.scalar.dma_start(dst, o_t[:])

        # software pipeline: prep one sample ahead so the next sample's
        # exp/loads are never queued behind this sample's evictions
        x0_t = x_pool.tile([P, KD, HW], F16, name="x0", tag="x")
        load_atts_and_x0(x0_t)
        staged = prep(0, x_t=x0_t, fine=True)
        for n in range(NPC):
            nxt = prep(n + 1) if n + 1 < NPC else None
            compute(n, *staged, last=(n == NPC - 1))
            staged = nxt

    nc.compile()
    return nc


_NC_CACHE = None


def _get_nc():
    global _NC_CACHE
    if _NC_CACHE is None:
        _NC_CACHE = build_nc()
    return _NC_CACHE


def run(in_maps, **kwargs):
    """Run the SPMD kernel on cores 0..7. in_maps: one dict per core."""
    nc = _get_nc()
    return run_bass_kernel_spmd(nc, in_maps, core_ids=list(range(NCORES)), **kwargs)


def make_in_maps(images: np.ndarray, atts: np.ndarray):
    images = np.ascontiguousarray(
        np.asarray(images, dtype=np.float32).astype(np.float16)
    )
    atts = np.asarray(atts, dtype=np.float32).astype(np.float16)
    assert images.shape == (N, C, H, W), images.shape
    assert atts.shape == (N, C, C), atts.shape
    img_s = images.reshape(NCORES, NPC, C, HW)
    a = atts.reshape(NCORES, NPC, C, C)
    # attsT[i][d, n, c] = atts[i][n, c, d]; attsC[i][c, n, d] = atts[i][n, c, d]
    attsT = np.ascontiguousarray(a.transpose(0, 3, 1, 2))
    attsC = np.ascontiguousarray(a.transpose(0, 2, 1, 3))
    return [
        {"images": np.ascontiguousarray(img_s[i]), "attsT": attsT[i], "attsC": attsC[i]}
        for i in range(NCORES)
    ]


def kernel(images: np.ndarray, atts: np.ndarray) -> np.ndarray:
    in_maps = make_in_maps(images, atts)
    res = run(in_maps)
    outs = [res.results[i]["out"] for i in range(NCORES)]
    full = np.concatenate(outs, axis=0).reshape(N, C, H, W)
    return full.astype(np.float32)


# revision 4
# speedup vs baseline: 1.1461x; 1.1461x over previous
"""Trainium2 Bass kernel for AttentionalPlanarRemapping.

out[n,c,h,w] = sum_d softmax(atts[n,c,:])[d] * images[n,d,h,w]

Per-sample: W = softmax(atts[n]) [C,C]; out[n] = W @ images[n].reshape(C, H*W).

Sharding: data-parallel over N across 8 cores (4 samples per core).

Host preprocessing inside kernel(): atts is passed TRANSPOSED per sample
(attsT[n] = atts[n].T, layout [d, c]) and cast to fp16: attsT loads with
the contraction dim d on partitions, which is exactly the matmul lhsT
layout, at half the DMA cost of f32. images are uploaded as fp16 and the
output is stored as fp16 (values only -- the returned array is float32).

Per-core plan, per sample (software-pipelined: prep(n+1) is emitted
before compute(n); prep uses only the sync/scalar queues so it can never
block compute's vector-engine evictions):
  prep(n):    DMA attsT[n] -> A [128, KD, 512] fp16, DMA images[n] ->
              X [128, KD, 1024] fp16; E = exp(A) fp16 (ACT, one instr;
              no max-sub: |atts| < 6 so exp is safe)
  compute(n): s_ps[p,c] = sum_d E[d,c] replicated across partitions
              (ones.T @ E, 4 PE matmuls); s_sb = fp16 copy (DVE); main
              matmuls kc=0..3 into ps[128,1024] (PSUM pool depth 3);
              after kc=0: rp redistribution (4 tiny PE matmuls
              s_sb-blk.T @ (1/128) -> [128, KC]) then r = 1/s (DVE);
              evict o = ps * r[:,kc] -> fp16 (DVE for kc<3, ACT kc==3);
              store per kc on alternating SWDGE (gpsimd) / HWDGE
              (scalar) queues.
Sample 0 is special-cased for ramp: loads are chunked per kd and
interleaved (attsT chunk, images chunk, ...), exp runs per chunk, and
the main kc=0 matmuls are emitted BEFORE the ones/rp denominator work so
the PE starts as soon as the first 128 rows land. The last sample's
final band is evicted and stored in halves on both engines/queues to
shrink the tail.
"""

import numpy as np
from contextlib import ExitStack

import concourse.bass as bass
import concourse.mybir as mybir
import concourse.tile as tile
from concourse import bacc
from concourse.bass_utils import run_bass_kernel_spmd

N, C, H, W = 32, 512, 32, 32
HW = H * W                      # 1024
NCORES = 8
NPC = N // NCORES               # 4 samples per core
P = 128
KC = C // P                     # 4 chunks over output channel c
KD = C // P                     # 4 chunks over contraction d
NT = 512                        # matmul moving free dim (one PSUM bank of f32)
NHT = HW // NT                  # 2

F32 = mybir.dt.float32
F16 = mybir.dt.float16
AF = mybir.ActivationFunctionType
AX = mybir.AxisListType


def build_nc():
    nc = bacc.Bacc("TRN2", target_bir_lowering=False, debug=False)

    images = nc.dram_tensor("images", [NPC, C, HW], F16, kind="ExternalInput").ap()
    attsT = nc.dram_tensor("attsT", [NPC, C, C], F16, kind="ExternalInput").ap()
    out = nc.dram_tensor("out", [NPC, C, HW], F16, kind="ExternalOutput").ap()

    with ExitStack() as ctx:
        tc = ctx.enter_context(tile.TileContext(nc))

        const_pool = ctx.enter_context(tc.tile_pool(name="const", bufs=1))
        ones = const_pool.tile([P, P], F16)
        oinv = const_pool.tile([P, 2], F16)

        a_pool = ctx.enter_context(tc.tile_pool(name="a", bufs=2))
        e_pool = ctx.enter_context(tc.tile_pool(name="e", bufs=2))
        x_pool = ctx.enter_context(tc.tile_pool(name="x", bufs=2))
        o_pool = ctx.enter_context(tc.tile_pool(name="o", bufs=6))
        st_pool = ctx.enter_context(tc.tile_pool(name="st", bufs=2))
        sm_psum = ctx.enter_context(tc.tile_pool(name="smp", bufs=1, space="PSUM"))
        mm_psum = ctx.enter_context(tc.tile_pool(name="mmp", bufs=3, space="PSUM"))

        def prep(n, fine=False):
            """Input DMAs + exp for sample n (sync + scalar queues only)."""
            a_t = a_pool.tile([P, KD, C], F16, name=f"a{n}", tag="a")
            x_t = x_pool.tile([P, KD, HW], F16, name=f"x{n}", tag="x")
            e_t = e_pool.tile([P, KD, C], F16, name=f"e{n}", tag="e")
            if fine:
                nc.gpsimd.memset(ones[:], 1.0)
                nc.gpsimd.memset(oinv[:], 1.0 / P)
                for kd in range(KD):
                    nc.sync.dma_start(a_t[:, kd], attsT[n][kd * P : (kd + 1) * P])
                    nc.sync.dma_start(x_t[:, kd], images[n][kd * P : (kd + 1) * P])
                for kd in range(KD):
                    nc.scalar.activation(
                        e_t[:, kd], a_t[:, kd], AF.Exp, bias=0.0, scale=1.0
                    )
            else:
                nc.sync.dma_start(
                    a_t[:], attsT[n].rearrange("(kd p) c -> p kd c", p=P)
                )
                nc.sync.dma_start(
                    x_t[:], images[n].rearrange("(kd p) f -> p kd f", p=P)
                )
                nc.scalar.activation(e_t[:], a_t[:], AF.Exp, bias=0.0, scale=1.0)
            return e_t, x_t

        def emit_ones(n, e_t):
            """Replicated denominators: s_ps[p, c] = sum_d E[d, c] (PE)."""
            s_ps = sm_psum.tile([P, C], F32, name=f"s{n}", tag="s", space="PSUM")
            for kd in range(KD):
                nc.tensor.matmul(
                    s_ps[:],
                    lhsT=ones[:],
                    rhs=e_t[:, kd],
                    start=(kd == 0),
                    stop=(kd == KD - 1),
                )
            s_sb = st_pool.tile([P, C], F16, name=f"ssb{n}", tag="ssb")
            nc.vector.tensor_copy(s_sb[:], s_ps[:])
            return s_sb

        def emit_rp(n, s_sb, r_t):
            """Redistribute s to per-partition layout via tiny PE matmuls,
            then r = 1/s on DVE."""
            rp_ps = sm_psum.tile([P, 2 * KC], F32, name=f"rp{n}", tag="rp", space="PSUM")
            for j in range(KC):
                nc.tensor.matmul(
                    rp_ps[:, j * 2 : (j + 1) * 2],
                    lhsT=s_sb[:, j * P : (j + 1) * P],
                    rhs=oinv[:],
                )
            s_col = st_pool.tile([P, KC], F32, name=f"scol{n}", tag="scol")
            nc.vector.tensor_copy(
                s_col[:],
                rp_ps[:].rearrange("p (kc j) -> p kc j", j=2)[:, :, 0],
            )
            nc.vector.reciprocal(r_t[:], s_col[:])

        def emit_mms(kc, e_t, x_t, ps):
            for kd in range(KD):
                lhs = e_t[:, kd, kc * P : (kc + 1) * P]
                for ht in range(NHT):
                    nc.tensor.matmul(
                        ps[:, ht * NT : (ht + 1) * NT],
                        lhsT=lhs,
                        rhs=x_t[:, kd, ht * NT : (ht + 1) * NT],
                        start=(kd == 0),
                        stop=(kd == KD - 1),
                    )

        def emit_evict(n, kc, ps, r_t, last):
            r_ap = r_t[:, kc : kc + 1]
            o_t = o_pool.tile([P, HW], F16, name=f"o{n}_{kc}", tag="o")
            dst = out[n][kc * P : (kc + 1) * P]
            if last and kc == KC - 1:
                # tail: evict + store the final band in halves on both
                # engines/queues so the kernel end isn't serialized behind
                # one full-width eviction
                nc.scalar.mul(o_t[:, 0:NT], ps[:, 0:NT], r_ap)
                nc.vector.tensor_scalar_mul(o_t[:, NT:HW], ps[:, NT:HW], r_ap)
                nc.gpsimd.dma_start(dst[:, 0:NT], o_t[:, 0:NT])
                nc.scalar.dma_start(dst[:, NT:HW], o_t[:, NT:HW])
            else:
                if kc == KC - 1:
                    nc.scalar.mul(o_t[:], ps[:], r_ap)
                else:
                    nc.vector.tensor_scalar_mul(o_t[:], ps[:], r_ap)
                if kc % 2 == 0:
                    nc.gpsimd.dma_start(dst, o_t[:])
                else:
                    nc.scalar.dma_start(dst, o_t[:])

        def compute(n, e_t, x_t, first=False, last=False):
            r_t = st_pool.tile([P, KC], F32, name=f"r{n}", tag="r")
            ps_tiles = {}
            if first:
                # ramp: main kc=0 matmuls first (they only need the first
                # e/x chunks), denominator work after
                ps_tiles[0] = mm_psum.tile([P, HW], F32, name=f"ps{n}_0", tag="ps", space="PSUM")
                emit_mms(0, e_t, x_t, ps_tiles[0])
                s_sb = emit_ones(n, e_t)
                kc_order = [1, 2, 3]
            else:
                s_sb = emit_ones(n, e_t)
                ps_tiles[0] = mm_psum.tile([P, HW], F32, name=f"ps{n}_0", tag="ps", space="PSUM")
                emit_mms(0, e_t, x_t, ps_tiles[0])
                kc_order = [1, 2, 3]
            emit_rp(n, s_sb, r_t)
            emit_evict(n, 0, ps_tiles[0], r_t, last)
            for kc in kc_order:
                ps = mm_psum.tile([P, HW], F32, name=f"ps{n}_{kc}", tag="ps", space="PSUM")
                emit_mms(kc, e_t, x_t, ps)
                emit_evict(n, kc, ps, r_t, last)

        # software pipeline: prep one sample ahead so the next sample's
        # exp/loads are never queued behind this sample's evictions
        staged = prep(0, fine=True)
        for n in range(NPC):
            nxt = prep(n + 1) if n + 1 < NPC else None
            compute(n, *staged, first=(n == 0), last=(n == NPC - 1))
            staged = nxt

    nc.compile()
    return nc


_NC_CACHE = None


def _get_nc():
    global _NC_CACHE
    if _NC_CACHE is None:
        _NC_CACHE = build_nc()
    return _NC_CACHE


def run(in_maps, **kwargs):
    """Run the SPMD kernel on cores 0..7. in_maps: one dict per core."""
    nc = _get_nc()
    return run_bass_kernel_spmd(nc, in_maps, core_ids=list(range(NCORES)), **kwargs)


def make_in_maps(images: np.ndarray, atts: np.ndarray):
    images = np.ascontiguousarray(
        np.asarray(images, dtype=np.float32).astype(np.float16)
    )
    atts = np.asarray(atts, dtype=np.float32)
    assert images.shape == (N, C, H, W), images.shape
    assert atts.shape == (N, C, C), atts.shape
    img_s = images.reshape(NCORES, NPC, C, HW)
    # per-sample transpose: attsT[n] = atts[n].T  (layout [d, c]), fp16
    attsT = np.ascontiguousarray(
        atts.transpose(0, 2, 1).astype(np.float16)
    ).reshape(NCORES, NPC, C, C)
    return [
        {"images": np.ascontiguousarray(img_s[i]), "attsT": attsT[i]}
        for i in range(NCORES)
    ]


def kernel(images: np.ndarray, atts: np.ndarray) -> np.ndarray:
    in_maps = make_in_maps(images, atts)
    res = run(in_maps)
    outs = [res.results[i]["out"] for i in range(NCORES)]
    full = np.concatenate(outs, axis=0).reshape(N, C, H, W)
    return full.astype(np.float32)


# revision 7
# speedup vs baseline: 1.1900x; 1.0382x over previous
"""Trainium2 Bass kernel for AttentionalPlanarRemapping.

out[n,c,h,w] = sum_d softmax(atts[n,c,:])[d] * images[n,d,h,w]

Per-sample: W = softmax(atts[n]) [C,C]; out[n] = W @ images[n].reshape(C, H*W).

Sharding: data-parallel over N across 8 cores (4 samples per core).

Host preprocessing inside kernel(): atts is passed TRANSPOSED per sample
(attsT[n] = atts[n].T, layout [d, c]) and cast to fp16: attsT loads with
the contraction dim d on partitions, which is exactly the matmul lhsT
layout, at half the DMA cost of f32. images are uploaded as fp16 and the
output is stored as fp16 (values only -- the returned array is float32).

Per-core plan, per sample (software-pipelined: prep(n+1) is emitted
before compute(n); prep uses only the sync/scalar queues so it can never
block compute's vector-engine evictions):
  prep(n):    DMA attsT[n] -> A [128, KD, 512] fp16, DMA images[n] ->
              X [128, KD, 1024] fp16; E = exp(A) fp16 (ACT, one instr;
              no max-sub: |atts| < 6 so exp is safe)
  compute(n): s_ps[p,c] = sum_d E[d,c] replicated across partitions
              (ones.T @ E, 4 PE matmuls); s_sb = fp16 copy (DVE); main
              matmuls kc=0..3 into ps[128,1024] (PSUM pool depth 3);
              after kc=0: rp redistribution (4 tiny PE matmuls
              s_sb-blk.T @ (1/128) -> [128, KC]) then r = 1/s (DVE);
              evict o = ps * r[:,kc] -> fp16 (DVE for kc<3, ACT kc==3);
              store per kc on alternating SWDGE (gpsimd) / HWDGE
              (scalar) queues.
Sample 0 is special-cased for ramp: loads are chunked per kd and
interleaved (attsT chunk, images chunk, ...), exp runs per chunk, and
the main kc=0 matmuls are emitted BEFORE the ones/rp denominator work so
the PE starts as soon as the first 128 rows land. The last sample's
final band is evicted and stored in halves on both engines/queues to
shrink the tail.
"""

import numpy as np
from contextlib import ExitStack

import concourse.bass as bass
import concourse.mybir as mybir
import concourse.tile as tile
from concourse import bacc
from concourse.bass_utils import run_bass_kernel_spmd

N, C, H, W = 32, 512, 32, 32
HW = H * W                      # 1024
NCORES = 8
NPC = N // NCORES               # 4 samples per core
P = 128
KC = C // P                     # 4 chunks over output channel c
KD = C // P                     # 4 chunks over contraction d
NT = 512                        # matmul moving free dim (one PSUM bank of f32)
NHT = HW // NT                  # 2

F32 = mybir.dt.float32
F16 = mybir.dt.float16
AF = mybir.ActivationFunctionType
AX = mybir.AxisListType


def build_nc():
    nc = bacc.Bacc("TRN2", target_bir_lowering=False, debug=False)

    images = nc.dram_tensor("images", [NPC, C, HW], F16, kind="ExternalInput").ap()
    attsT = nc.dram_tensor("attsT", [NPC, C, C], F16, kind="ExternalInput").ap()
    out = nc.dram_tensor("out", [NPC, C, HW], F16, kind="ExternalOutput").ap()

    with ExitStack() as ctx:
        tc = ctx.enter_context(tile.TileContext(nc))

        const_pool = ctx.enter_context(tc.tile_pool(name="const", bufs=1))
        ones = const_pool.tile([P, P], F16)
        oinv = const_pool.tile([P, 2], F16)

        a_pool = ctx.enter_context(tc.tile_pool(name="a", bufs=2))
        e_pool = ctx.enter_context(tc.tile_pool(name="e", bufs=2))
        x_pool = ctx.enter_context(tc.tile_pool(name="x", bufs=2))
        o_pool = ctx.enter_context(tc.tile_pool(name="o", bufs=6))
        st_pool = ctx.enter_context(tc.tile_pool(name="st", bufs=2))
        sm_psum = ctx.enter_context(tc.tile_pool(name="smp", bufs=1, space="PSUM"))
        mm_psum = ctx.enter_context(tc.tile_pool(name="mmp", bufs=3, space="PSUM"))

        def prep(n, fine=False):
            """Input DMAs + exp for sample n (sync + scalar queues only)."""
            a_t = a_pool.tile([P, KD, C], F16, name=f"a{n}", tag="a")
            x_t = x_pool.tile([P, KD, HW], F16, name=f"x{n}", tag="x")
            e_t = e_pool.tile([P, KD, C], F16, name=f"e{n}", tag="e")
            if fine:
                nc.gpsimd.memset(ones[:], 1.0)
                nc.gpsimd.memset(oinv[:], 1.0 / P)
                for kd in range(KD):
                    nc.sync.dma_start(a_t[:, kd], attsT[n][kd * P : (kd + 1) * P])
                    nc.sync.dma_start(x_t[:, kd], images[n][kd * P : (kd + 1) * P])
                for kd in range(KD):
                    nc.scalar.activation(
                        e_t[:, kd], a_t[:, kd], AF.Exp, bias=0.0, scale=1.0
                    )
            else:
                nc.sync.dma_start(
                    a_t[:], attsT[n].rearrange("(kd p) c -> p kd c", p=P)
                )
                nc.sync.dma_start(
                    x_t[:], images[n].rearrange("(kd p) f -> p kd f", p=P)
                )
                nc.scalar.activation(e_t[:], a_t[:], AF.Exp, bias=0.0, scale=1.0)
            return e_t, x_t

        def emit_e2(n, e_t):
            """Pre-sum E over kd on DVE (f16, 2x rate) so the replicated
            denominator needs only ONE ones-matmul on the PE."""
            e2a = st_pool.tile([P, C], F16, name=f"e2a{n}", tag="e2a")
            nc.vector.tensor_add(e2a[:], e_t[:, 0], e_t[:, 1])
            e2b = st_pool.tile([P, C], F16, name=f"e2b{n}", tag="e2b")
            nc.vector.tensor_add(e2b[:], e_t[:, 2], e_t[:, 3])
            e2 = st_pool.tile([P, C], F16, name=f"e2{n}", tag="e2")
            nc.vector.tensor_add(e2[:], e2a[:], e2b[:])
            return e2

        def emit_ones(n, e2):
            """Replicated denominators: s_ps[p, c] = sum_d E[d, c] (PE)."""
            s_ps = sm_psum.tile([P, C], F32, name=f"s{n}", tag="s", space="PSUM")
            nc.tensor.matmul(s_ps[:], lhsT=ones[:], rhs=e2[:])
            s_sb = st_pool.tile([P, C], F16, name=f"ssb{n}", tag="ssb")
            nc.vector.tensor_copy(s_sb[:], s_ps[:])
            return s_sb

        def emit_rp(n, s_sb, r_t):
            """Redistribute s to per-partition layout via tiny PE matmuls,
            then r = 1/s on DVE."""
            rp_ps = sm_psum.tile([P, 2 * KC], F32, name=f"rp{n}", tag="rp", space="PSUM")
            for j in range(KC):
                nc.tensor.matmul(
                    rp_ps[:, j * 2 : (j + 1) * 2],
                    lhsT=s_sb[:, j * P : (j + 1) * P],
                    rhs=oinv[:],
                )
            s_col = st_pool.tile([P, KC], F32, name=f"scol{n}", tag="scol")
            nc.vector.tensor_copy(
                s_col[:],
                rp_ps[:].rearrange("p (kc j) -> p kc j", j=2)[:, :, 0],
            )
            nc.vector.reciprocal(r_t[:], s_col[:])

        def emit_mms(kc, e_t, x_t, ps, ht_major=False):
            if ht_major:
                # ht-major: the first half's accumulation group completes
                # 4 matmuls early, so its eviction overlaps the second
                # half's matmuls (used for the very last band)
                for ht in range(NHT):
                    for kd in range(KD):
                        nc.tensor.matmul(
                            ps[:, ht * NT : (ht + 1) * NT],
                            lhsT=e_t[:, kd, kc * P : (kc + 1) * P],
                            rhs=x_t[:, kd, ht * NT : (ht + 1) * NT],
                            start=(kd == 0),
                            stop=(kd == KD - 1),
                        )
            else:
                for kd in range(KD):
                    lhs = e_t[:, kd, kc * P : (kc + 1) * P]
                    for ht in range(NHT):
                        nc.tensor.matmul(
                            ps[:, ht * NT : (ht + 1) * NT],
                            lhsT=lhs,
                            rhs=x_t[:, kd, ht * NT : (ht + 1) * NT],
                            start=(kd == 0),
                            stop=(kd == KD - 1),
                        )

        def emit_evict(n, kc, ps, r_t, engine):
            r_ap = r_t[:, kc : kc + 1]
            o_t = o_pool.tile([P, HW], F16, name=f"o{n}_{kc}", tag="o")
            dst = out[n][kc * P : (kc + 1) * P]
            if engine == "vector":
                nc.vector.tensor_scalar_mul(o_t[:], ps[:], r_ap)
            else:
                nc.scalar.mul(o_t[:], ps[:], r_ap)
            if kc % 2 == 0:
                nc.gpsimd.dma_start(dst, o_t[:])
            else:
                nc.scalar.dma_start(dst, o_t[:])

        def emit_last_band(n, kc, e_t, x_t, ps, r_t):
            """Final band: ht-major matmuls; each half is evicted into its
            own half tile as soon as its accumulation group completes and
            stored immediately (parallel engines + queues) to minimize the
            kernel tail."""
            emit_mms(kc, e_t, x_t, ps, ht_major=True)
            r_ap = r_t[:, kc : kc + 1]
            dst = out[n][kc * P : (kc + 1) * P]
            o_a = o_pool.tile([P, NT], F16, name=f"oa{n}", tag="oa")
            nc.scalar.mul(o_a[:], ps[:, 0:NT], r_ap)
            nc.gpsimd.dma_start(dst[:, 0:NT], o_a[:])
            o_b = o_pool.tile([P, NT], F16, name=f"ob{n}", tag="ob")
            nc.vector.tensor_scalar_mul(o_b[:], ps[:, NT:HW], r_ap)
            nc.scalar.dma_start(dst[:, NT:HW], o_b[:])

        # eviction engine per kc: DVE first (its queue is free of prep
        # work), ACT for the later bands (after exp(n+1) has drained)
        EV_ENGINE = ["vector", "vector", "scalar", "scalar"]

        def compute(n, e_t, x_t, e2, first=False, last=False, nxt_e=None):
            r_t = st_pool.tile([P, KC], F32, name=f"r{n}", tag="r")
            ps_tiles = {}
            if first:
                # ramp: main kc=0 matmuls first (they only need the first
                # e/x chunks), denominator work after
                ps_tiles[0] = mm_psum.tile([P, HW], F32, name=f"ps{n}_0", tag="ps", space="PSUM")
                emit_mms(0, e_t, x_t, ps_tiles[0])
                s_sb = emit_ones(n, e2)
            else:
                s_sb = emit_ones(n, e2)
                ps_tiles[0] = mm_psum.tile([P, HW], F32, name=f"ps{n}_0", tag="ps", space="PSUM")
                emit_mms(0, e_t, x_t, ps_tiles[0])
            emit_rp(n, s_sb, r_t)
            emit_evict(n, 0, ps_tiles[0], r_t, EV_ENGINE[0])
            nxt_e2 = None
            for kc in range(1, KC):
                ps = mm_psum.tile([P, HW], F32, name=f"ps{n}_{kc}", tag="ps", space="PSUM")
                if last and kc == KC - 1:
                    emit_last_band(n, kc, e_t, x_t, ps, r_t)
                else:
                    emit_mms(kc, e_t, x_t, ps)
                    emit_evict(n, kc, ps, r_t, EV_ENGINE[kc])
                if kc == 1 and nxt_e is not None:
                    # deferred: next sample's E2 adds go on the vector queue
                    # AFTER this sample's first evictions so they can never
                    # delay a PSUM slot release
                    nxt_e2 = emit_e2(n + 1, nxt_e)
            return nxt_e2

        # software pipeline: prep one sample ahead so the next sample's
        # exp/loads are never queued behind this sample's evictions
        e0_t, x0_t = prep(0, fine=True)
        e2_cur = emit_e2(0, e0_t)
        staged = (e0_t, x0_t)
        for n in range(NPC):
            nxt = prep(n + 1) if n + 1 < NPC else None
            e2_cur = compute(
                n,
                *staged,
                e2_cur,
                first=(n == 0),
                last=(n == NPC - 1),
                nxt_e=(nxt[0] if nxt is not None else None),
            )
            staged = nxt

    nc.compile()
    return nc


_NC_CACHE = None


def _get_nc():
    global _NC_CACHE
    if _NC_CACHE is None:
        _NC_CACHE = build_nc()
    return _NC_CACHE


def run(in_maps, **kwargs):
    """Run the SPMD kernel on cores 0..7. in_maps: one dict per core."""
    nc = _get_nc()
    return run_bass_kernel_spmd(nc, in_maps, core_ids=list(range(NCORES)), **kwargs)


def make_in_maps(images: np.ndarray, atts: np.ndarray):
    images = np.ascontiguousarray(
        np.asarray(images, dtype=np.float32).astype(np.float16)
    )
    atts = np.asarray(atts, dtype=np.float32)
    assert images.shape == (N, C, H, W), images.shape
    assert atts.shape == (N, C, C), atts.shape
    img_s = images.reshape(NCORES, NPC, C, HW)
    # per-sample transpose: attsT[n] = atts[n].T  (layout [d, c]), fp16
    attsT = np.ascontiguousarray(
        atts.transpose(0, 2, 1).astype(np.float16)
    ).reshape(NCORES, NPC, C, C)
    return [
        {"images": np.ascontiguousarray(img_s[i]), "attsT": attsT[i]}
        for i in range(NCORES)
    ]


def kernel(images: np.ndarray, atts: np.ndarray) -> np.ndarray:
    in_maps = make_in_maps(images, atts)
    res = run(in_maps)
    outs = [res.results[i]["out"] for i in range(NCORES)]
    full = np.concatenate(outs, axis=0).reshape(N, C, H, W)
    return full.astype(np.float32)
